# revision 5
# baseline (speedup 1.0000x reference)
"""Trainium2 Bass kernel for nn_BackBone_77532749627801.

Transformer encoder block: per-head QKV projections -> MHA (softmax over
keys) -> AddNorm -> FFN (erf GELU) -> AddNorm.  B=4, S=2048, D=1024, H=16,
DH=64, F=4096.

Sharding: 8 cores = 4 batches x 2 sequence-halves.  Each core computes the
block for 1024 query tokens of one batch; K/V are computed for the full
2048-token sequence on both cores of a batch, which removes every
collective.  Per-core query selection is done host-side by rotating each
core's xT so its own tokens occupy columns 0:1024.

Precision plan (gate is 2e-2 rel err; measured headroom on the real data):
QKV projections and attn@V run in fp8e4m3 with DoubleRow perf mode (0.5
cycles/row, 2 contraction chunks per pass); attention scores and both FFN
matmuls run in bf16 (1 cycle/row); fp8 on the FFN was measured at 3.8e-2
end-to-end and is excluded.  All PSUM accumulation is fp32; layernorms and
residuals are fp32.  The softmax denominator falls out of the attn@V matmul
via an appended ones-column on V; QK biases are applied by the
activation-engine eviction; softmax uses a constant shift (scores are
O(+-6)).  The FFN intermediate (gelu activations) stays resident in SBUF as
bf16 instead of round-tripping 33 MB through DRAM, and LN2 + output DMA are
fused per 512-token group into the FFN2 loop.
"""

import contextlib
import os
import sys

import numpy as np

if "/opt/trn_rl_repo" not in sys.path and os.path.isdir("/opt/trn_rl_repo"):
    sys.path.insert(0, "/opt/trn_rl_repo")

B, S, D, H, DH, F = 4, 2048, 1024, 16, 64, 4096
N_CORES = 8
TOK = 1024  # query tokens per core
EPS = 1e-5
EXP_SHIFT = -3.0  # constant shift inside exp; cancels in softmax

_BUILD_CACHE = {}


def _build(n_iters=1):
    import concourse.bacc as bacc
    import concourse.mybir as mybir
    import concourse.tile as tile
    from concourse.masks import make_identity
    from contextlib import ExitStack

    f32 = mybir.dt.float32
    bf16 = mybir.dt.bfloat16
    e4 = mybir.dt.float8e4
    AF = mybir.ActivationFunctionType
    DR = mybir.MatmulPerfMode.DoubleRow

    nc = bacc.Bacc("TRN2", target_bir_lowering=False, debug=False,
                   num_devices=N_CORES)

    xT = nc.dram_tensor("xT", [D, S], e4, kind="ExternalInput").ap()
    xh = nc.dram_tensor("xh", [TOK, D], f32, kind="ExternalInput").ap()
    wq = nc.dram_tensor("wq", [D, D], e4, kind="ExternalInput").ap()
    wk = nc.dram_tensor("wk", [D, D], e4, kind="ExternalInput").ap()
    wv = nc.dram_tensor("wv", [D, 4, 264], e4, kind="ExternalInput").ap()
    bqk = nc.dram_tensor("bqk", [2, D], f32, kind="ExternalInput").ap()
    bv4 = nc.dram_tensor("bv4", [1, 4, 264], f32, kind="ExternalInput").ap()
    w1 = nc.dram_tensor("w1", [D, F], bf16, kind="ExternalInput").ap()
    b1d = nc.dram_tensor("b1", [F], f32, kind="ExternalInput").ap()
    w2 = nc.dram_tensor("w2", [F, D], bf16, kind="ExternalInput").ap()
    b2r = nc.dram_tensor("b2r", [1, D], f32, kind="ExternalInput").ap()
    ln1g = nc.dram_tensor("ln1g", [D], f32, kind="ExternalInput").ap()
    ln2g = nc.dram_tensor("ln2g", [D], f32, kind="ExternalInput").ap()
    ln2b = nc.dram_tensor("ln2b", [D], f32, kind="ExternalInput").ap()
    out = nc.dram_tensor("out", [TOK, D], f32, kind="ExternalOutput").ap()

    with tile.TileContext(nc) as tc, ExitStack() as top:
        const = top.enter_context(tc.tile_pool(name="const", bufs=1))
        ident_f = const.tile([128, 128], f32)
        make_identity(nc, ident_f)
        ident_h = const.tile([128, 128], bf16)
        make_identity(nc, ident_h)
        eshift = const.tile([128, 1], f32)
        nc.vector.memset(eshift, EXP_SHIFT)
        eps_t = const.tile([128, 1], f32)
        nc.vector.memset(eps_t, EPS)
        bq_sb = const.tile([128, 8], f32)
        nc.sync.dma_start(out=bq_sb, in_=bqk[0].rearrange("(pr p) -> p pr", p=128))
        bk_sb = const.tile([128, 8], f32)
        nc.sync.dma_start(out=bk_sb, in_=bqk[1].rearrange("(pr p) -> p pr", p=128))
        b1_sb = const.tile([128, 32], f32)
        nc.sync.dma_start(out=b1_sb, in_=b1d.rearrange("(fc p) -> p fc", p=128))

        resid = top.enter_context(tc.tile_pool(name="resid", bufs=1))

        loop = tc.For_i(0, n_iters) if n_iters > 1 else contextlib.nullcontext()
        with loop:
            mha = resid.tile([128, 8, D], f32, tag="mha")

            # residual input + LN constants: DMA'd up front so phase B never
            # waits on them (they overlap all of phase A)
            xh_sb = resid.tile([128, 8, D], f32, tag="xh")
            for st in range(8):
                nc.sync.dma_start(
                    out=xh_sb[:, st, :],
                    in_=xh[st * 128:(st + 1) * 128, :])
            g1_bc = resid.tile([128, D], f32, tag="g1")
            nc.gpsimd.dma_start(out=g1_bc, in_=ln1g.partition_broadcast(128))
            g2_bc = resid.tile([128, D], f32, tag="g2")
            nc.gpsimd.dma_start(out=g2_bc, in_=ln2g.partition_broadcast(128))
            b2_bc = resid.tile([128, D], f32, tag="lb2")
            nc.gpsimd.dma_start(out=b2_bc, in_=ln2b.partition_broadcast(128))
            b2row = resid.tile([1, D], bf16, tag="b2row")
            nc.sync.dma_start(out=b2row, in_=b2rh[:, :])

            # ---------------- Phase A: QKV + attention ----------------
            with ExitStack() as pha:
                xpool = pha.enter_context(tc.tile_pool(name="xT", bufs=1))
                wp = pha.enter_context(tc.tile_pool(name="wpair", bufs=2))
                att = pha.enter_context(tc.tile_pool(name="att", bufs=1))
                zpool = pha.enter_context(tc.tile_pool(name="zp", bufs=2))
                psA = pha.enter_context(
                    tc.tile_pool(name="psA", bufs=1, space="PSUM"))

                xT_sb = xpool.tile([128, 8, S], e4)
                nc.sync.dma_start(
                    out=xT_sb, in_=xT.rearrange("(dt p) s -> p dt s", p=128))

                for quad in range(4):
                    # V + bias (+ ones cols) for 4 heads: vplus[t, 65j+e]
                    wv_sb = wp.tile([128, 8, 264], e4, tag="wv")
                    nc.sync.dma_start(
                        out=wv_sb,
                        in_=wv[:, quad, :].rearrange("(dt p) c -> p dt c", p=128))
                    bv_bc = wp.tile([128, 264], f32, tag="bv_bc")
                    nc.gpsimd.dma_start(
                        out=bv_bc, in_=bv4[0, quad, :].partition_broadcast(128))
                    vplus = att.tile([128, 16, 264], e4, tag="vplus")
                    for tt in range(16):
                        vp_ps = psA.tile([128, 264], f32, tag="small", bufs=2)
                        for dp in range(4):
                            nc.tensor.matmul(
                                out=vp_ps,
                                lhsT=xT_sb[:, 2 * dp:2 * dp + 2,
                                           tt * 128:(tt + 1) * 128],
                                rhs=wv_sb[:, 2 * dp:2 * dp + 2, :],
                                start=(dp == 0), stop=(dp == 3),
                                perf_mode=DR)
                        nc.vector.tensor_add(out=vplus[:, tt, :], in0=vp_ps,
                                             in1=bv_bc[:, :])

                    for pr01 in range(2):
                        pair = 2 * quad + pr01
                        wk_sb = wp.tile([128, 8, 128], e4, tag="wk")
                        nc.sync.dma_start(
                            out=wk_sb,
                            in_=wk[:, pair * 128:(pair + 1) * 128].rearrange(
                                "(dt p) m -> p dt m", p=128))
                        wq_sb = wp.tile([128, 8, 128], e4, tag="wq")
                        nc.sync.dma_start(
                            out=wq_sb,
                            in_=wq[:, pair * 128:(pair + 1) * 128].rearrange(
                                "(dt p) m -> p dt m", p=128))

                        kT = att.tile([128, S], bf16, tag="kT")
                        for ch in range(2):
                            kq_ps = psA.tile([128, 1024], f32, tag="sT2", bufs=2)
                            for nh in range(2):
                                for dp in range(4):
                                    nc.tensor.matmul(
                                        out=kq_ps[:, nh * 512:(nh + 1) * 512],
                                        lhsT=wk_sb[:, 2 * dp:2 * dp + 2, :],
                                        rhs=xT_sb[:, 2 * dp:2 * dp + 2,
                                                  (2 * ch + nh) * 512:
                                                  (2 * ch + nh + 1) * 512],
                                        start=(dp == 0), stop=(dp == 3),
                                        perf_mode=DR)
                            nc.scalar.activation(
                                out=kT[:, ch * 1024:(ch + 1) * 1024], in_=kq_ps,
                                func=AF.Identity, bias=bk_sb[:, pair:pair + 1])

                        # own tokens are xT columns 0:1024 (host rotation)
                        qT = att.tile([128, TOK], bf16, tag="qT")
                        kq_ps = psA.tile([128, 1024], f32, tag="sT2", bufs=2)
                        for nh in range(2):
                            for dp in range(4):
                                nc.tensor.matmul(
                                    out=kq_ps[:, nh * 512:(nh + 1) * 512],
                                    lhsT=wq_sb[:, 2 * dp:2 * dp + 2, :],
                                    rhs=xT_sb[:, 2 * dp:2 * dp + 2,
                                              nh * 512:(nh + 1) * 512],
                                    start=(dp == 0), stop=(dp == 3),
                                    perf_mode=DR)
                        nc.scalar.activation(
                            out=qT[:, :], in_=kq_ps,
                            func=AF.Identity, bias=bq_sb[:, pair:pair + 1])

                        for h01 in range(2):
                            head = 2 * pair + h01
                            j = 2 * pr01 + h01
                            pslice = slice(h01 * 64, h01 * 64 + 64)
                            for sch in range(2):
                                expT = att.tile([128, 16, 512], e4,
                                                tag="expT", bufs=2)
                                for tp in range(8):
                                    sT_ps = psA.tile([128, 1024], f32,
                                                     tag="sT2", bufs=2)
                                    for sub in range(2):
                                        tt = 2 * tp + sub
                                        nc.tensor.matmul(
                                            out=sT_ps[:, sub * 512:
                                                      (sub + 1) * 512],
                                            lhsT=kT[pslice,
                                                    tt * 128:(tt + 1) * 128],
                                            rhs=qT[pslice,
                                                   sch * 512:(sch + 1) * 512],
                                            start=True, stop=True)
                                    nc.scalar.activation(
                                        out=expT[:, 2 * tp:2 * tp + 2, :],
                                        in_=sT_ps, func=AF.Exp,
                                        bias=eshift[:, :], scale=0.125)
                                zT_ps = psA.tile([66, 512], f32, tag="zT",
                                                 bufs=2)
                                for t2 in range(8):
                                    nc.tensor.matmul(
                                        out=zT_ps,
                                        lhsT=vplus[:, 2 * t2:2 * t2 + 2,
                                                   66 * j:66 * j + 66],
                                        rhs=expT[:, 2 * t2:2 * t2 + 2, :],
                                        start=(t2 == 0), stop=(t2 == 7),
                                        perf_mode=DR)
                                zT_sb = zpool.tile([66, 512], bf16,
                                                   tag="zT_sb")
                                nc.vector.tensor_copy(out=zT_sb, in_=zT_ps)
                                for sb4 in range(4):
                                    ztr = psA.tile([128, 66], bf16,
                                                   tag="small", bufs=2)
                                    nc.tensor.transpose(
                                        out=ztr,
                                        in_=zT_sb[:, sb4 * 128:(sb4 + 1) * 128],
                                        identity=ident_h[0:66, 0:66])
                                    rec = zpool.tile([128, 1], f32, tag="rec")
                                    nc.vector.reciprocal(
                                        out=rec, in_=ztr[:, 64:65])
                                    stg = sch * 4 + sb4
                                    nc.vector.tensor_scalar_mul(
                                        out=mha[:, stg,
                                                head * 64:head * 64 + 64],
                                        in0=ztr[:, 0:64], scalar1=rec)

            # ---------------- Phase B: AddNorm1 + FFN + AddNorm2 --------
            with ExitStack() as phb:
                bpool = phb.enter_context(tc.tile_pool(name="bpool", bufs=1))
                stream = phb.enter_context(tc.tile_pool(name="stream", bufs=4))
                stat = phb.enter_context(tc.tile_pool(name="stat", bufs=4))

                xh_sb = bpool.tile([128, 8, D], f32, tag="xh")
                for st in range(8):
                    nc.sync.dma_start(
                        out=xh_sb[:, st, :],
                        in_=xh[st * 128:(st + 1) * 128, :])
                ffb2_bc = bpool.tile([128, D], f32, tag="b2bc")
                nc.gpsimd.dma_start(out=ffb2_bc,
                                    in_=b2r[0, :].partition_broadcast(128))
                g1_bc = bpool.tile([128, D], f32, tag="g1")
                nc.gpsimd.dma_start(out=g1_bc, in_=ln1g.partition_broadcast(128))
                g2_bc = bpool.tile([128, D], f32, tag="g2")
                nc.gpsimd.dma_start(out=g2_bc, in_=ln2g.partition_broadcast(128))
                b2_bc = bpool.tile([128, D], f32, tag="lb2")
                nc.gpsimd.dma_start(out=b2_bc, in_=ln2b.partition_broadcast(128))

                def layer_norm_tile(st, g_bc, b_bc, add_in=None, add_ps=None,
                                    out_dma=False):
                    h = mha[:, st, :]
                    if add_in is not None:
                        nc.vector.tensor_add(out=h, in0=h, in1=add_in)
                    if add_ps is not None:
                        nc.vector.tensor_add(out=h, in0=h, in1=add_ps)
                        nc.vector.tensor_add(out=h, in0=h, in1=ffb2_bc[:, :])
                    stats = stat.tile([128, 2, 6], f32, tag="stats")
                    for sg in range(2):
                        nc.vector.bn_stats(
                            out=stats[:, sg, :],
                            in_=h[:, sg * 512:(sg + 1) * 512])
                    mv = stat.tile([128, 2], f32, tag="mv")
                    nc.vector.bn_aggr(out=mv, in_=stats)
                    nc.scalar.activation(
                        out=mv[:, 1:2], in_=mv[:, 1:2],
                        func=AF.Sqrt, bias=eps_t[:, :])
                    nc.vector.reciprocal(out=mv[:, 1:2], in_=mv[:, 1:2])
                    nc.vector.tensor_scalar(
                        out=h, in0=h, scalar1=mv[:, 0:1],
                        scalar2=mv[:, 1:2],
                        op0=mybir.AluOpType.subtract,
                        op1=mybir.AluOpType.mult)
                    if g_bc is not None:
                        nc.vector.tensor_mul(out=h, in0=h, in1=g_bc[:, :])
                        nc.vector.tensor_add(out=h, in0=h, in1=b_bc[:, :])
                    if out_dma:
                        nc.sync.dma_start(
                            out=out.rearrange(
                                "(st p) d -> p st d", p=128)[:, st, :],
                            in_=h)

                # LN1 gamma/beta are folded host-side into W1/b1 (FFN path)
                # and into b2r (residual path); apply only the normalize here
                # so h1T/FFN1 can start sooner.  mha holds y = normalized.
                for st in range(8):
                    layer_norm_tile(st, None, None, add_in=xh_sb[:, st, :])

                # h1T[d, s] in bf16 for the FFN1 matmul
                h1T = bpool.tile([128, 8, TOK], bf16, tag="h1T")
                with tc.tile_pool(name="psT", bufs=1, space="PSUM") as psT:
                    for st in range(8):
                        for dt in range(8):
                            tr_ps = psT.tile([128, 128], f32, tag="tr", bufs=3)
                            nc.tensor.transpose(
                                out=tr_ps,
                                in_=mha[:, st, dt * 128:(dt + 1) * 128],
                                identity=ident_f[:, :])
                            nc.vector.tensor_copy(
                                out=h1T[:, dt, st * 128:(st + 1) * 128],
                                in_=tr_ps)

                # residual stream: h1 = y*g1 + (ln1_b folded into b2r)
                for st in range(8):
                    nc.vector.tensor_mul(out=mha[:, st, :], in0=mha[:, st, :],
                                         in1=g1_bc[:, :])

                # FFN pass 1: aT[fc] = gelu(W1^T h1 + b1), SBUF-resident bf16
                aT = bpool.tile([128, 32, TOK], bf16, tag="aT")
                with tc.tile_pool(name="ps1", bufs=1, space="PSUM") as ps1:
                    for fc in range(32):
                        w1t = stream.tile([128, 8, 128], bf16, tag="w1t")
                        nc.sync.dma_start(
                            out=w1t,
                            in_=w1[:, fc * 128:(fc + 1) * 128].rearrange(
                                "(dt p) f -> p dt f", p=128))
                        a_ps = ps1.tile([128, TOK], f32, tag="aps", bufs=2)
                        for nh in range(2):
                            for dt in range(8):
                                nc.tensor.matmul(
                                    out=a_ps[:, nh * 512:(nh + 1) * 512],
                                    lhsT=w1t[:, dt, :],
                                    rhs=h1T[:, dt, nh * 512:(nh + 1) * 512],
                                    start=(dt == 0), stop=(dt == 7))
                        nc.scalar.activation(
                            out=aT[:, fc, :], in_=a_ps, func=AF.Gelu,
                            bias=b1_sb[:, fc:fc + 1])

                # FFN pass 2: ff = aT^T @ W2 + b2; then fused AddNorm2 + DMA
                with tc.tile_pool(name="ps2", bufs=1, space="PSUM") as ps2:
                    for sh in range(2):
                        ffps = [ps2.tile([128, D], f32, tag="ff", bufs=4,
                                         name=f"ff_{sh}_{i}")
                                for i in range(4)]
                        for fc in range(32):
                            w2t = stream.tile([128, D], bf16, tag="w2t")
                            nc.sync.dma_start(
                                out=w2t, in_=w2[fc * 128:(fc + 1) * 128, :])
                            for st2 in range(4):
                                base = sh * 512 + st2 * 128
                                for nh in range(2):
                                    nc.tensor.matmul(
                                        out=ffps[st2][:, nh * 512:(nh + 1) * 512],
                                        lhsT=aT[:, fc, base:base + 128],
                                        rhs=w2t[:, nh * 512:(nh + 1) * 512],
                                        start=(fc == 0), stop=(fc == 31))
                        for st2 in range(4):
                            layer_norm_tile(sh * 4 + st2, g2_bc, b2_bc,
                                            add_ps=ffps[st2][:, :],
                                            out_dma=True)

    nc.compile()
    return nc


def _pack_inputs(x, Wq, bq, Wk, bk, Wv, bv, ln1_g, ln1_b, W1, b1, W2, b2,
                 ln2_g, ln2_b):
    """Build the 8 per-core input maps (host-side, numpy)."""
    from concourse import mybir

    f = np.float32
    E4 = mybir.dt.np(mybir.dt.float8e4)
    BF = mybir.dt.np(mybir.dt.bfloat16)
    wq_all = np.ascontiguousarray(
        np.transpose(np.asarray(Wq, f), (1, 0, 2)).reshape(D, D)).astype(E4)
    wk_all = np.ascontiguousarray(
        np.transpose(np.asarray(Wk, f), (1, 0, 2)).reshape(D, D)).astype(E4)
    Wv_ = np.asarray(Wv, f)
    bv_ = np.asarray(bv, f)
    wv_all = np.zeros((D, 4, 264), f)
    bv_all = np.zeros((1, 4, 264), f)
    for quad in range(4):
        for j in range(4):
            h = quad * 4 + j
            wv_all[:, quad, 66 * j:66 * j + 64] = Wv_[h]
            bv_all[0, quad, 66 * j:66 * j + 64] = bv_[h]
            bv_all[0, quad, 66 * j + 64] = 1.0
    bqk = np.stack([np.asarray(bq, f).reshape(D), np.asarray(bk, f).reshape(D)])
    x = np.asarray(x, f)
    W1_ = np.asarray(W1, np.float64)
    g1_ = np.asarray(ln1_g, np.float64)
    bb1_ = np.asarray(ln1_b, np.float64)
    w1_folded = (g1_[:, None] * W1_).astype(BF)
    b1_folded = (np.asarray(b1, np.float64) + bb1_ @ W1_).astype(f)
    b2_folded = (np.asarray(b2, np.float64) + bb1_).astype(f)
    common = dict(
        wq=wq_all, wk=wk_all, wv=wv_all.astype(E4), bqk=bqk, bv4=bv_all,
        w1=w1_folded, b1=b1_folded, w2=np.asarray(W2, f).astype(BF),
        b2r=b2_folded.reshape(1, D),
        ln1g=np.asarray(ln1_g, f),
        ln2g=np.asarray(ln2_g, f), ln2b=np.asarray(ln2_b, f))
    in_maps = []
    for c in range(N_CORES):
        b_, half = c // 2, c % 2
        m = dict(common)
        own = x[b_, half * TOK:(half + 1) * TOK]
        other = x[b_, (1 - half) * TOK:(2 - half) * TOK]
        m["xT"] = np.ascontiguousarray(
            np.concatenate([own, other], axis=0).T).astype(E4)
        m["xh"] = np.ascontiguousarray(own)
        in_maps.append(m)
    return in_maps


def kernel(**inputs):
    from concourse.bass_utils import run_bass_kernel_spmd

    if "nc" not in _BUILD_CACHE:
        _BUILD_CACHE["nc"] = _build()
    nc = _BUILD_CACHE["nc"]
    in_maps = _pack_inputs(**inputs)
    res = run_bass_kernel_spmd(nc, in_maps, core_ids=list(range(N_CORES)))
    out = np.zeros((B, S, D), np.float32)
    for c in range(N_CORES):
        b_, half = c // 2, c % 2
        out[b_, half * TOK:(half + 1) * TOK] = res.results[c]["out"]
    return out


# revision 8
# speedup vs baseline: 1.4466x; 1.4466x over previous
"""Trainium2 Bass kernel for nn_BackBone_77532749627801.

Transformer encoder block: per-head QKV projections -> MHA (softmax over
keys) -> AddNorm -> FFN (erf GELU) -> AddNorm.  B=4, S=2048, D=1024, H=16,
DH=64, F=4096.

Sharding: 8 cores = 4 batches x 2 sequence-halves.  Each core computes the
block for 1024 query tokens of one batch; K/V are computed for the full
2048-token sequence on both cores of a batch, which removes every
collective.  Per-core query selection is done host-side by rotating each
core's xT so its own tokens occupy columns 0:1024.

Precision plan (gate is 2e-2 rel err; measured headroom on the real data):
QKV projections and attn@V run in fp8e4m3 with DoubleRow perf mode (0.5
cycles/row, 2 contraction chunks per pass); attention scores and both FFN
matmuls run in bf16 (1 cycle/row); fp8 on the FFN was measured at 3.8e-2
end-to-end and is excluded.  All PSUM accumulation is fp32; layernorms and
residuals are fp32.  The softmax denominator falls out of the attn@V matmul
via an appended ones-column on V; QK biases are applied by the
activation-engine eviction; softmax uses a constant shift (scores are
O(+-6)).  The FFN intermediate (gelu activations) stays resident in SBUF as
bf16 instead of round-tripping 33 MB through DRAM, and LN2 + output DMA are
fused per 512-token group into the FFN2 loop.
"""

import contextlib
import os
import sys

import numpy as np

if "/opt/trn_rl_repo" not in sys.path and os.path.isdir("/opt/trn_rl_repo"):
    sys.path.insert(0, "/opt/trn_rl_repo")

B, S, D, H, DH, F = 4, 2048, 1024, 16, 64, 4096
N_CORES = 8
TOK = 1024  # query tokens per core
EPS = 1e-5
EXP_SHIFT = -3.0  # constant shift inside exp; cancels in softmax

_BUILD_CACHE = {}


def _build(n_iters=1):
    import concourse.bacc as bacc
    import concourse.mybir as mybir
    import concourse.tile as tile
    from concourse.masks import make_identity
    from contextlib import ExitStack

    f32 = mybir.dt.float32
    bf16 = mybir.dt.bfloat16
    e4 = mybir.dt.float8e4
    AF = mybir.ActivationFunctionType
    DR = mybir.MatmulPerfMode.DoubleRow

    nc = bacc.Bacc("TRN2", target_bir_lowering=False, debug=False,
                   num_devices=N_CORES)

    xT = nc.dram_tensor("xT", [D, S], e4, kind="ExternalInput").ap()
    xh = nc.dram_tensor("xh", [TOK, D], f32, kind="ExternalInput").ap()
    wq = nc.dram_tensor("wq", [D, D], e4, kind="ExternalInput").ap()
    wk = nc.dram_tensor("wk", [D, D], e4, kind="ExternalInput").ap()
    wv = nc.dram_tensor("wv", [D, 4, 264], e4, kind="ExternalInput").ap()
    bqk = nc.dram_tensor("bqk", [2, D], f32, kind="ExternalInput").ap()
    bv4 = nc.dram_tensor("bv4", [1, 4, 264], f32, kind="ExternalInput").ap()
    w1 = nc.dram_tensor("w1", [D, F], bf16, kind="ExternalInput").ap()
    b1d = nc.dram_tensor("b1", [F], f32, kind="ExternalInput").ap()
    w2 = nc.dram_tensor("w2", [F, D], bf16, kind="ExternalInput").ap()
    b2rh = nc.dram_tensor("b2rh", [1, D], bf16, kind="ExternalInput").ap()
    ln1g = nc.dram_tensor("ln1g", [D], f32, kind="ExternalInput").ap()
    ln2g = nc.dram_tensor("ln2g", [D], f32, kind="ExternalInput").ap()
    ln2b = nc.dram_tensor("ln2b", [D], f32, kind="ExternalInput").ap()
    out = nc.dram_tensor("out", [TOK, D], f32, kind="ExternalOutput").ap()

    with tile.TileContext(nc) as tc, ExitStack() as top:
        const = top.enter_context(tc.tile_pool(name="const", bufs=1))
        ident_f = const.tile([128, 128], f32)
        make_identity(nc, ident_f)
        ident_h = const.tile([128, 128], bf16)
        make_identity(nc, ident_h)
        eshift = const.tile([128, 1], f32)
        nc.vector.memset(eshift, EXP_SHIFT)
        eps_t = const.tile([128, 1], f32)
        nc.vector.memset(eps_t, EPS)
        bq_sb = const.tile([128, 8], f32)
        nc.sync.dma_start(out=bq_sb, in_=bqk[0].rearrange("(pr p) -> p pr", p=128))
        bk_sb = const.tile([128, 8], f32)
        nc.sync.dma_start(out=bk_sb, in_=bqk[1].rearrange("(pr p) -> p pr", p=128))
        b1_sb = const.tile([128, 32], f32)
        nc.sync.dma_start(out=b1_sb, in_=b1d.rearrange("(fc p) -> p fc", p=128))
        ones_r = const.tile([1, 128], bf16)
        nc.vector.memset(ones_r, 1.0)

        resid = top.enter_context(tc.tile_pool(name="resid", bufs=1))

        loop = tc.For_i(0, n_iters) if n_iters > 1 else contextlib.nullcontext()
        with loop:
            mha = resid.tile([128, 8, D], f32, tag="mha")

            # residual input + LN constants: DMA'd up front so phase B never
            # waits on them (they overlap all of phase A)
            xh_sb = resid.tile([128, 8, D], f32, tag="xh")
            for st in range(8):
                nc.sync.dma_start(
                    out=xh_sb[:, st, :],
                    in_=xh[st * 128:(st + 1) * 128, :])
            g1_bc = resid.tile([128, D], f32, tag="g1")
            nc.gpsimd.dma_start(out=g1_bc, in_=ln1g.partition_broadcast(128))
            g2_bc = resid.tile([128, D], f32, tag="g2")
            nc.gpsimd.dma_start(out=g2_bc, in_=ln2g.partition_broadcast(128))
            b2_bc = resid.tile([128, D], f32, tag="lb2")
            nc.gpsimd.dma_start(out=b2_bc, in_=ln2b.partition_broadcast(128))
            b2row = resid.tile([1, D], bf16, tag="b2row")
            nc.sync.dma_start(out=b2row, in_=b2rh[:, :])

            # ---------------- Phase A: QKV + attention ----------------
            with ExitStack() as pha:
                xpool = pha.enter_context(tc.tile_pool(name="xT", bufs=1))
                wp = pha.enter_context(tc.tile_pool(name="wpair", bufs=2))
                att = pha.enter_context(tc.tile_pool(name="att", bufs=1))
                zpool = pha.enter_context(tc.tile_pool(name="zp", bufs=2))
                psA = pha.enter_context(
                    tc.tile_pool(name="psA", bufs=1, space="PSUM"))

                xT_sb = xpool.tile([128, 8, S], e4)
                nc.sync.dma_start(
                    out=xT_sb, in_=xT.rearrange("(dt p) s -> p dt s", p=128))

                for quad in range(4):
                    # V + bias (+ ones cols) for 4 heads: vplus[t, 65j+e]
                    wv_sb = wp.tile([128, 8, 264], e4, tag="wv")
                    nc.sync.dma_start(
                        out=wv_sb,
                        in_=wv[:, quad, :].rearrange("(dt p) c -> p dt c", p=128))
                    bv_bc = wp.tile([128, 264], f32, tag="bv_bc")
                    nc.gpsimd.dma_start(
                        out=bv_bc, in_=bv4[0, quad, :].partition_broadcast(128))
                    vplus = att.tile([128, 16, 264], e4, tag="vplus")
                    for tt in range(16):
                        vp_ps = psA.tile([128, 264], f32, tag="small", bufs=2)
                        for dp in range(4):
                            nc.tensor.matmul(
                                out=vp_ps,
                                lhsT=xT_sb[:, 2 * dp:2 * dp + 2,
                                           tt * 128:(tt + 1) * 128],
                                rhs=wv_sb[:, 2 * dp:2 * dp + 2, :],
                                start=(dp == 0), stop=(dp == 3),
                                perf_mode=DR)
                        nc.vector.tensor_add(out=vplus[:, tt, :], in0=vp_ps,
                                             in1=bv_bc[:, :])

                    for pr01 in range(2):
                        pair = 2 * quad + pr01
                        wk_sb = wp.tile([128, 8, 128], e4, tag="wk")
                        nc.sync.dma_start(
                            out=wk_sb,
                            in_=wk[:, pair * 128:(pair + 1) * 128].rearrange(
                                "(dt p) m -> p dt m", p=128))
                        wq_sb = wp.tile([128, 8, 128], e4, tag="wq")
                        nc.sync.dma_start(
                            out=wq_sb,
                            in_=wq[:, pair * 128:(pair + 1) * 128].rearrange(
                                "(dt p) m -> p dt m", p=128))

                        kT = att.tile([128, S], bf16, tag="kT")
                        for ch in range(2):
                            kq_ps = psA.tile([128, 1024], f32, tag="sT2", bufs=2)
                            for nh in range(2):
                                for dp in range(4):
                                    nc.tensor.matmul(
                                        out=kq_ps[:, nh * 512:(nh + 1) * 512],
                                        lhsT=wk_sb[:, 2 * dp:2 * dp + 2, :],
                                        rhs=xT_sb[:, 2 * dp:2 * dp + 2,
                                                  (2 * ch + nh) * 512:
                                                  (2 * ch + nh + 1) * 512],
                                        start=(dp == 0), stop=(dp == 3),
                                        perf_mode=DR)
                            nc.scalar.activation(
                                out=kT[:, ch * 1024:(ch + 1) * 1024], in_=kq_ps,
                                func=AF.Identity, bias=bk_sb[:, pair:pair + 1])

                        # own tokens are xT columns 0:1024 (host rotation)
                        qT = att.tile([128, TOK], bf16, tag="qT")
                        kq_ps = psA.tile([128, 1024], f32, tag="sT2", bufs=2)
                        for nh in range(2):
                            for dp in range(4):
                                nc.tensor.matmul(
                                    out=kq_ps[:, nh * 512:(nh + 1) * 512],
                                    lhsT=wq_sb[:, 2 * dp:2 * dp + 2, :],
                                    rhs=xT_sb[:, 2 * dp:2 * dp + 2,
                                              nh * 512:(nh + 1) * 512],
                                    start=(dp == 0), stop=(dp == 3),
                                    perf_mode=DR)
                        nc.scalar.activation(
                            out=qT[:, :], in_=kq_ps,
                            func=AF.Identity, bias=bq_sb[:, pair:pair + 1])

                        for h01 in range(2):
                            head = 2 * pair + h01
                            j = 2 * pr01 + h01
                            pslice = slice(h01 * 64, h01 * 64 + 64)
                            for sch in range(2):
                                expT = att.tile([128, 16, 512], e4,
                                                tag="expT", bufs=2)
                                for tp in range(8):
                                    sT_ps = psA.tile([128, 1024], f32,
                                                     tag="sT2", bufs=2)
                                    for sub in range(2):
                                        tt = 2 * tp + sub
                                        nc.tensor.matmul(
                                            out=sT_ps[:, sub * 512:
                                                      (sub + 1) * 512],
                                            lhsT=kT[pslice,
                                                    tt * 128:(tt + 1) * 128],
                                            rhs=qT[pslice,
                                                   sch * 512:(sch + 1) * 512],
                                            start=True, stop=True)
                                    nc.scalar.activation(
                                        out=expT[:, 2 * tp:2 * tp + 2, :],
                                        in_=sT_ps, func=AF.Exp,
                                        bias=eshift[:, :], scale=0.125)
                                zT_ps = psA.tile([66, 512], f32, tag="zT",
                                                 bufs=2)
                                for t2 in range(8):
                                    nc.tensor.matmul(
                                        out=zT_ps,
                                        lhsT=vplus[:, 2 * t2:2 * t2 + 2,
                                                   66 * j:66 * j + 66],
                                        rhs=expT[:, 2 * t2:2 * t2 + 2, :],
                                        start=(t2 == 0), stop=(t2 == 7),
                                        perf_mode=DR)
                                zT_sb = zpool.tile([66, 512], bf16,
                                                   tag="zT_sb")
                                nc.vector.tensor_copy(out=zT_sb, in_=zT_ps)
                                for sb4 in range(4):
                                    ztr = psA.tile([128, 66], bf16,
                                                   tag="small", bufs=2)
                                    nc.tensor.transpose(
                                        out=ztr,
                                        in_=zT_sb[:, sb4 * 128:(sb4 + 1) * 128],
                                        identity=ident_h[0:66, 0:66])
                                    rec = zpool.tile([128, 1], f32, tag="rec")
                                    nc.vector.reciprocal(
                                        out=rec, in_=ztr[:, 64:65])
                                    stg = sch * 4 + sb4
                                    nc.vector.tensor_scalar_mul(
                                        out=mha[:, stg,
                                                head * 64:head * 64 + 64],
                                        in0=ztr[:, 0:64], scalar1=rec)

            # ---------------- Phase B: AddNorm1 + FFN + AddNorm2 --------
            with ExitStack() as phb:
                bpool = phb.enter_context(tc.tile_pool(name="bpool", bufs=1))
                stream = phb.enter_context(tc.tile_pool(name="stream", bufs=4))
                stat = phb.enter_context(tc.tile_pool(name="stat", bufs=4))

                def layer_norm_tile(st, g_bc, b_bc, add_in=None, add_ps=None,
                                    out_dma=False):
                    h = mha[:, st, :]
                    if add_in is not None:
                        nc.vector.tensor_add(out=h, in0=h, in1=add_in)
                    if add_ps is not None:
                        nc.vector.tensor_add(out=h, in0=h, in1=add_ps)
                    stats = stat.tile([128, 2, 6], f32, tag="stats")
                    for sg in range(2):
                        nc.vector.bn_stats(
                            out=stats[:, sg, :],
                            in_=h[:, sg * 512:(sg + 1) * 512])
                    mv = stat.tile([128, 2], f32, tag="mv")
                    nc.vector.bn_aggr(out=mv, in_=stats)
                    nc.scalar.activation(
                        out=mv[:, 1:2], in_=mv[:, 1:2],
                        func=AF.Sqrt, bias=eps_t[:, :])
                    nc.vector.reciprocal(out=mv[:, 1:2], in_=mv[:, 1:2])
                    nc.vector.tensor_scalar(
                        out=h, in0=h, scalar1=mv[:, 0:1],
                        scalar2=mv[:, 1:2],
                        op0=mybir.AluOpType.subtract,
                        op1=mybir.AluOpType.mult)
                    if g_bc is not None:
                        nc.vector.tensor_mul(out=h, in0=h, in1=g_bc[:, :])
                        nc.vector.tensor_add(out=h, in0=h, in1=b_bc[:, :])
                    if out_dma:
                        nc.sync.dma_start(
                            out=out.rearrange(
                                "(st p) d -> p st d", p=128)[:, st, :],
                            in_=h)

                # LN1 gamma/beta are folded host-side into W1/b1 (FFN path)
                # and into b2r (residual path); apply only the normalize here
                # so h1T/FFN1 can start sooner.  mha holds y = normalized.
                for st in range(8):
                    layer_norm_tile(st, None, None, add_in=xh_sb[:, st, :])

                # h1T[d, s] in bf16 for the FFN1 matmul
                h1T = bpool.tile([128, 8, TOK], bf16, tag="h1T")
                with tc.tile_pool(name="psT", bufs=1, space="PSUM") as psT:
                    for st in range(8):
                        for dt in range(8):
                            tr_ps = psT.tile([128, 128], f32, tag="tr", bufs=3)
                            nc.tensor.transpose(
                                out=tr_ps,
                                in_=mha[:, st, dt * 128:(dt + 1) * 128],
                                identity=ident_f[:, :])
                            nc.vector.tensor_copy(
                                out=h1T[:, dt, st * 128:(st + 1) * 128],
                                in_=tr_ps)

                # residual stream: h1 = y*g1 + (ln1_b folded into b2r)
                for st in range(8):
                    nc.vector.tensor_mul(out=mha[:, st, :], in0=mha[:, st, :],
                                         in1=g1_bc[:, :])

                # FFN pass 1: aT[fc] = gelu(W1^T h1 + b1), SBUF-resident bf16
                aT = bpool.tile([128, 32, TOK], bf16, tag="aT")
                with tc.tile_pool(name="ps1", bufs=1, space="PSUM") as ps1:
                    for fc in range(32):
                        w1t = stream.tile([128, 8, 128], bf16, tag="w1t")
                        nc.sync.dma_start(
                            out=w1t,
                            in_=w1[:, fc * 128:(fc + 1) * 128].rearrange(
                                "(dt p) f -> p dt f", p=128))
                        a_ps = ps1.tile([128, TOK], f32, tag="aps", bufs=2)
                        for nh in range(2):
                            for dt in range(8):
                                nc.tensor.matmul(
                                    out=a_ps[:, nh * 512:(nh + 1) * 512],
                                    lhsT=w1t[:, dt, :],
                                    rhs=h1T[:, dt, nh * 512:(nh + 1) * 512],
                                    start=(dt == 0), stop=(dt == 7))
                        nc.scalar.activation(
                            out=aT[:, fc, :], in_=a_ps, func=AF.Gelu,
                            bias=b1_sb[:, fc:fc + 1])

                # FFN pass 2: ff = aT^T @ W2 + b2; then fused AddNorm2 + DMA
                with tc.tile_pool(name="ps2", bufs=1, space="PSUM") as ps2:
                    for sh in range(2):
                        ffps = [ps2.tile([128, D], f32, tag="ff", bufs=4,
                                         name=f"ff_{sh}_{i}")
                                for i in range(4)]
                        for fc in range(32):
                            w2t = stream.tile([128, D], bf16, tag="w2t")
                            nc.sync.dma_start(
                                out=w2t, in_=w2[fc * 128:(fc + 1) * 128, :])
                            for st2 in range(4):
                                base = sh * 512 + st2 * 128
                                for nh in range(2):
                                    nc.tensor.matmul(
                                        out=ffps[st2][:, nh * 512:(nh + 1) * 512],
                                        lhsT=aT[:, fc, base:base + 128],
                                        rhs=w2t[:, nh * 512:(nh + 1) * 512],
                                        start=(fc == 0), stop=False)
                        # + b2 (with folded ln1_b) via rank-1 ones matmul;
                        # closes each accumulation group
                        for st2 in range(4):
                            for nh in range(2):
                                nc.tensor.matmul(
                                    out=ffps[st2][:, nh * 512:(nh + 1) * 512],
                                    lhsT=ones_r[:, :],
                                    rhs=b2row[:, nh * 512:(nh + 1) * 512],
                                    start=False, stop=True)
                        for st2 in range(4):
                            layer_norm_tile(sh * 4 + st2, g2_bc, b2_bc,
                                            add_ps=ffps[st2][:, :],
                                            out_dma=True)

    nc.compile()
    return nc


def _pack_inputs(x, Wq, bq, Wk, bk, Wv, bv, ln1_g, ln1_b, W1, b1, W2, b2,
                 ln2_g, ln2_b):
    """Build the 8 per-core input maps (host-side, numpy)."""
    from concourse import mybir

    f = np.float32
    E4 = mybir.dt.np(mybir.dt.float8e4)
    BF = mybir.dt.np(mybir.dt.bfloat16)
    wq_all = np.ascontiguousarray(
        np.transpose(np.asarray(Wq, f), (1, 0, 2)).reshape(D, D)).astype(E4)
    wk_all = np.ascontiguousarray(
        np.transpose(np.asarray(Wk, f), (1, 0, 2)).reshape(D, D)).astype(E4)
    Wv_ = np.asarray(Wv, f)
    bv_ = np.asarray(bv, f)
    wv_all = np.zeros((D, 4, 264), f)
    bv_all = np.zeros((1, 4, 264), f)
    for quad in range(4):
        for j in range(4):
            h = quad * 4 + j
            wv_all[:, quad, 66 * j:66 * j + 64] = Wv_[h]
            bv_all[0, quad, 66 * j:66 * j + 64] = bv_[h]
            bv_all[0, quad, 66 * j + 64] = 1.0
    bqk = np.stack([np.asarray(bq, f).reshape(D), np.asarray(bk, f).reshape(D)])
    x = np.asarray(x, f)
    W1_ = np.asarray(W1, np.float64)
    g1_ = np.asarray(ln1_g, np.float64)
    bb1_ = np.asarray(ln1_b, np.float64)
    w1_folded = (g1_[:, None] * W1_).astype(BF)
    b1_folded = (np.asarray(b1, np.float64) + bb1_ @ W1_).astype(f)
    b2_folded = (np.asarray(b2, np.float64) + bb1_).astype(f)
    common = dict(
        wq=wq_all, wk=wk_all, wv=wv_all.astype(E4), bqk=bqk, bv4=bv_all,
        w1=w1_folded, b1=b1_folded, w2=np.asarray(W2, f).astype(BF),
        b2rh=b2_folded.reshape(1, D).astype(BF),
        ln1g=np.asarray(ln1_g, f),
        ln2g=np.asarray(ln2_g, f), ln2b=np.asarray(ln2_b, f))
    in_maps = []
    for c in range(N_CORES):
        b_, half = c // 2, c % 2
        m = dict(common)
        own = x[b_, half * TOK:(half + 1) * TOK]
        other = x[b_, (1 - half) * TOK:(2 - half) * TOK]
        m["xT"] = np.ascontiguousarray(
            np.concatenate([own, other], axis=0).T).astype(E4)
        m["xh"] = np.ascontiguousarray(own)
        in_maps.append(m)
    return in_maps


def kernel(**inputs):
    from concourse.bass_utils import run_bass_kernel_spmd

    if "nc" not in _BUILD_CACHE:
        _BUILD_CACHE["nc"] = _build()
    nc = _BUILD_CACHE["nc"]
    in_maps = _pack_inputs(**inputs)
    res = run_bass_kernel_spmd(nc, in_maps, core_ids=list(range(N_CORES)))
    out = np.zeros((B, S, D), np.float32)
    for c in range(N_CORES):
        b_, half = c // 2, c % 2
        out[b_, half * TOK:(half + 1) * TOK] = res.results[c]["out"]
    return out
